# revision 9
# baseline (speedup 1.0000x reference)
"""Trainium2 Bass kernel for the ActorCriticCriterion (AIC) masked REINFORCE loss.

Reference computation (per the oracle):
    at_or_after_eos = cumsum(seq == 0, axis=1) > 0
    seq_z  = where(at_or_after_eos, 0, seq)
    mask   = concat([ones(B,1), (seq_z > 0)[:, :-1]], axis=1)
    loss   = sum(-logp * (reward - value) * mask) / sum(mask)

Identity used: mask[t] = AND(seq[0..t-1] != 0) with mask[0] = 1 — computed
directly with one DVE tensor_tensor_scan (op0=logical_and) per 128-row group,
writing to a shifted access pattern (the leading ones column is a memset).

fp16 pipeline (streaming is the roofline — ~41us of HBM traffic at the
~410 GB/s/core measured rate; all compute must hide under it):
    DMA:    all input tiles stream on the sync HWDGE ring, pre-issued with
            fully-resident staging (no buffer-recycle stalls -> no gaps).
    ACT:    lp16/val16/rew16 = cast(f32 -> fp16).  Scalar engine is otherwise
            idle; fp16 operands let every DVE tensor_tensor run in 2x_1P
            perf mode (2 elem/lane/cyc needs all-2-byte operands) and every
            matmul at 1 cycle/row (fp32 is 4).
    DVE:    mask via logical_and scan (fp16 out; scans have no 2x mode),
            d = val16 - rew16, q = lp16 * d, mq = q * mask   (all 2x)
    PE:     ones16[128,1].T @ {mask,mq} chunks, everything accumulated into
            two single-bank PSUM accumulators num/den [1, 512] f32.
Outputs are the two [1, 512] accumulators; the host sums them and divides.
Sharding: pure data-parallel over B across 8 cores (1024 rows each).
"""

import os
import numpy as np

B, T = 8192, 1024
NCORES = 8
ROWS = B // NCORES          # rows per core
P = 128                     # SBUF partitions
MMCHUNK = 512               # matmul free-dim chunk (one PSUM bank)

_CACHE: dict = {}


def _build_program(rows: int):
    """Build the Bass/Tile program for one core processing `rows` rows."""
    from contextlib import ExitStack

    import concourse.bacc as bacc
    import concourse.mybir as mybir
    import concourse.tile as tile

    f32 = mybir.dt.float32
    f16 = mybir.dt.float16
    i32 = mybir.dt.int32
    Op = mybir.AluOpType

    ablk = int(os.environ.get("K_A", "1"))     # row-groups per sub-block
    nsub = rows // (P * ablk)
    assert nsub * P * ablk == rows

    # Bacc (not raw Bass): its compile pipeline splits multi-sem sync waits
    # into event-semaphore instructions — this walrus build allows at most
    # one wait per engine instruction.
    nc = bacc.Bacc()
    seq = nc.dram_tensor("seq", [rows, T], i32, kind="ExternalInput")
    lp = nc.dram_tensor("lp", [rows, T], f32, kind="ExternalInput")
    val = nc.dram_tensor("val", [rows, T], f32, kind="ExternalInput")
    rew = nc.dram_tensor("rew", [rows, T], f32, kind="ExternalInput")
    out_num = nc.dram_tensor("out_num", [1, MMCHUNK], f32,
                             kind="ExternalOutput")
    out_den = nc.dram_tensor("out_den", [1, MMCHUNK], f32,
                             kind="ExternalOutput")

    def dram_sub(t, r0, na):
        # rows [r0, r0 + na*P) as [p, a, t] with row = r0 + a*P + p
        return t[r0:r0 + na * P, :].rearrange("(a p) t -> p a t", p=P)

    light_tail = bool(int(os.environ.get("K_LIGHT_TAIL", "1")))
    ring_split = bool(int(os.environ.get("K_RING_SPLIT", "0")))

    with ExitStack() as ctx:
        tc = ctx.enter_context(tile.TileContext(nc))
        if light_tail:
            # Replace Tile's end-of-kernel epilogue (drain + two all-engine
            # EVSEM barriers + 64-sem clear, ~8-9us) with just the final
            # drain. Safe for re-execution: the Bass preamble dma_reset +
            # sem_clear runs at the START of every execution, so leaving
            # semaphores dirty at kernel end is fine.
            import types

            from concourse.vector_clock import ScopedClock

            def _light_drain_and_barrier(self, tick_clock, wait_clock):
                drain_inst = self.nc.sync.drain()
                wait_clock.add_sem_waits(
                    drain_inst.ins,
                    ScopedClock({None: tick_clock.global_clock}))
                popped = self.nc._tile_sem_poison_stack.pop()
                assert popped is self._sem_poison
                # Deliberately do NOT free the tile sems: Bacc's
                # event-semaphore pass allocates from the free pool after
                # this and must not alias sems still used by the kernel.

            tc._drain_and_barrier = types.MethodType(
                _light_drain_and_barrier, tc)
        const_pool = ctx.enter_context(tc.tile_pool(name="const", bufs=1))
        # One staging buffer per sub-block: the DMA ring never waits on a
        # buffer free, so the stream runs gap-free at line rate.
        in_pool = ctx.enter_context(tc.tile_pool(name="in", bufs=nsub))
        h_pool = ctx.enter_context(tc.tile_pool(name="h", bufs=2))
        scr_pool = ctx.enter_context(tc.tile_pool(name="scr", bufs=2))
        psum_pool = ctx.enter_context(
            tc.tile_pool(name="psum", bufs=1, space="PSUM"))

        ones16 = const_pool.tile([P, 1], f16)
        nc.vector.memset(ones16[:], 1.0)

        num_ps = psum_pool.tile([1, MMCHUNK], f32)
        den_ps = psum_pool.tile([1, MMCHUNK], f32)

        for si in range(nsub):
            r0 = si * P * ablk
            na = ablk
            seq_t = in_pool.tile([P, na, T], i32, tag="seq")
            lp_t = in_pool.tile([P, na, T], f32, tag="lp")
            val_t = in_pool.tile([P, na, T], f32, tag="val")
            rew_t = in_pool.tile([P, na, T], f32, tag="rew")
            # Single sync HWDGE ring carries the whole input stream (issue
            # order = arrival order); the scalar engine stays free for casts.
            # seq first (the scan is the longest DVE op), then val/rew (d),
            # then lp (q).
            eng2 = nc.scalar if ring_split else nc.sync
            last = si == nsub - 1
            nc.sync.dma_start(out=seq_t[:], in_=dram_sub(seq, r0, na))
            nc.sync.dma_start(out=val_t[:], in_=dram_sub(val, r0, na))
            eng2.dma_start(out=rew_t[:], in_=dram_sub(rew, r0, na))
            if last:
                # lp of the last sub-block is the final stream data: land it
                # in two halves so q/mq on the first half overlap the second
                # half's transfer.
                Ht = T // 2
                lp_dram = dram_sub(lp, r0, na)
                eng2.dma_start(out=lp_t[:, :, 0:Ht], in_=lp_dram[:, :, 0:Ht])
                eng2.dma_start(out=lp_t[:, :, Ht:], in_=lp_dram[:, :, Ht:])
            else:
                eng2.dma_start(out=lp_t[:], in_=dram_sub(lp, r0, na))

            # f32 -> fp16 casts on the (otherwise idle) Activation engine.
            # The last sub-block's lp is the final DMA of the whole stream:
            # skip its cast (q then reads lp as f32 at 1x) to drop a
            # sem-wait + cast hop from the data-end -> output critical path.
            lp16 = None if last else h_pool.tile([P, na, T], f16, tag="lp16")
            val16 = h_pool.tile([P, na, T], f16, tag="val16")
            rew16 = h_pool.tile([P, na, T], f16, tag="rew16")
            nc.scalar.copy(val16[:], val_t[:])
            nc.scalar.copy(rew16[:], rew_t[:])
            if not last:
                nc.scalar.copy(lp16[:], lp_t[:])

            # mask[p,a,0] = 1; mask[p,a,t] = AND(seq[p,a,0..t-1] != 0)
            mask = scr_pool.tile([P, na, T], f16, tag="mask", bufs=3)
            nc.vector.memset(mask[:, :, 0:1], 1.0)
            for a in range(na):
                nc.vector.tensor_tensor_scan(
                    out=mask[:, a, 1:T], data0=seq_t[:, a, 0:T - 1],
                    data1=seq_t[:, a, 0:T - 1], initial=1.0,
                    op0=Op.logical_and, op1=Op.bypass)

            # den column sums can go to PE as soon as the mask exists.
            for a in range(na):
                for c in range(0, T, MMCHUNK):
                    nc.tensor.matmul(
                        out=den_ps[:], lhsT=ones16[:],
                        rhs=mask[:, a, c:c + MMCHUNK],
                        start=(si == 0 and a == 0 and c == 0),
                        stop=(si == nsub - 1 and a == na - 1
                              and c == T - MMCHUNK))

            # d = val - rew ; q = logp * d ; mq = q * mask   (all fp16, 2x)
            d16 = scr_pool.tile([P, na, T], f16, tag="d")
            nc.vector.tensor_tensor(out=d16[:], in0=val16[:], in1=rew16[:],
                                    op=Op.subtract)
            q16 = scr_pool.tile([P, na, T], f16, tag="q")
            mq = scr_pool.tile([P, na, T], f16, tag="mq", bufs=3)
            if last:
                # Per-half q -> mq -> matmul chains so the first half's tail
                # work overlaps the second half's DMA.
                Ht = T // 2
                for h0 in (0, Ht):
                    hs = slice(h0, h0 + Ht)
                    nc.vector.tensor_tensor(out=q16[:, :, hs],
                                            in0=lp_t[:, :, hs],
                                            in1=d16[:, :, hs], op=Op.mult)
                    nc.vector.tensor_tensor(out=mq[:, :, hs],
                                            in0=q16[:, :, hs],
                                            in1=mask[:, :, hs], op=Op.mult)
                    for a in range(na):
                        for c in range(h0, h0 + Ht, MMCHUNK):
                            nc.tensor.matmul(
                                out=num_ps[:], lhsT=ones16[:],
                                rhs=mq[:, a, c:c + MMCHUNK],
                                start=False,
                                stop=(a == na - 1 and c == T - MMCHUNK))
            else:
                nc.vector.tensor_tensor(out=q16[:], in0=lp16[:], in1=d16[:],
                                        op=Op.mult)
                nc.vector.tensor_tensor(out=mq[:], in0=q16[:], in1=mask[:],
                                        op=Op.mult)
                for a in range(na):
                    for c in range(0, T, MMCHUNK):
                        nc.tensor.matmul(
                            out=num_ps[:], lhsT=ones16[:],
                            rhs=mq[:, a, c:c + MMCHUNK],
                            start=(si == 0 and a == 0 and c == 0),
                            stop=False)

        # PSUM can't be DMA'd directly — bounce through SBUF.  den finishes
        # first (its last matmul precedes num's), so its copy + store overlap
        # the num tail; the two copies run on different engines in parallel.
        num_sb = const_pool.tile([1, MMCHUNK], f32)
        den_sb = const_pool.tile([1, MMCHUNK], f32)
        Hc = MMCHUNK // 2
        nc.scalar.copy(den_sb[:], den_ps[:])
        nc.sync.dma_start(out=out_den[:], in_=den_sb[:])
        nc.vector.tensor_copy(num_sb[:, 0:Hc], num_ps[:, 0:Hc])
        nc.scalar.copy(num_sb[:, Hc:], num_ps[:, Hc:])
        nc.sync.dma_start(out=out_num[:], in_=num_sb[:])

    nc.finalize()
    return nc


def kernel(sample_seq, sample_seqLogprobs, sample_value, sample_reward):
    from concourse.bass_utils import run_bass_kernel_spmd

    seq = np.ascontiguousarray(np.asarray(sample_seq, dtype=np.int32))
    lp = np.ascontiguousarray(np.asarray(sample_seqLogprobs, dtype=np.float32))
    val = np.ascontiguousarray(np.asarray(sample_value, dtype=np.float32))
    rew = np.ascontiguousarray(np.asarray(sample_reward, dtype=np.float32))
    assert seq.shape == (B, T)

    if "nc" not in _CACHE:
        _CACHE["nc"] = _build_program(ROWS)
    nc = _CACHE["nc"]

    in_maps = []
    for c in range(NCORES):
        sl = slice(c * ROWS, (c + 1) * ROWS)
        in_maps.append({
            "seq": seq[sl], "lp": lp[sl], "val": val[sl], "rew": rew[sl],
        })

    trace = bool(int(os.environ.get("K_TRACE", "0")))
    res = run_bass_kernel_spmd(nc, in_maps, core_ids=list(range(NCORES)),
                               trace=trace)
    if trace:
        _CACHE["exec_time_ns"] = res.exec_time_ns
        _CACHE["trace"] = res.instructions_and_trace
    num = 0.0
    den = 0.0
    for r in res.results:
        num += float(np.asarray(r["out_num"], dtype=np.float64).sum())
        den += float(np.asarray(r["out_den"], dtype=np.float64).sum())
    return np.float32(num / den)


# revision 12
# speedup vs baseline: 1.0771x; 1.0771x over previous
"""Trainium2 Bass kernel for the ActorCriticCriterion (AIC) masked REINFORCE loss.

Reference computation (per the oracle):
    at_or_after_eos = cumsum(seq == 0, axis=1) > 0
    seq_z  = where(at_or_after_eos, 0, seq)
    mask   = concat([ones(B,1), (seq_z > 0)[:, :-1]], axis=1)
    loss   = sum(-logp * (reward - value) * mask) / sum(mask)

Identity used: mask[t] = AND(seq[0..t-1] != 0) with mask[0] = 1 — computed
directly with one DVE tensor_tensor_scan (op0=logical_and) per 128-row group,
writing to a shifted access pattern (the leading ones column is a memset).

fp16 pipeline (streaming is the roofline — ~41us of HBM traffic at the
~410 GB/s/core measured rate; all compute must hide under it):
    DMA:    all input tiles stream on the sync HWDGE ring, pre-issued with
            fully-resident staging (no buffer-recycle stalls -> no gaps).
    ACT:    lp16/val16/rew16 = cast(f32 -> fp16).  Scalar engine is otherwise
            idle; fp16 operands let every DVE tensor_tensor run in 2x_1P
            perf mode (2 elem/lane/cyc needs all-2-byte operands) and every
            matmul at 1 cycle/row (fp32 is 4).
    DVE:    mask via logical_and scan (fp16 out; scans have no 2x mode),
            d = val16 - rew16, q = lp16 * d, mq = q * mask   (all 2x)
    PE:     ones16[128,1].T @ {mask,mq} chunks, everything accumulated into
            two single-bank PSUM accumulators num/den [1, 512] f32.
Outputs are the two [1, 512] accumulators; the host sums them and divides.
Sharding: pure data-parallel over B across 8 cores (1024 rows each).
"""

import os
import numpy as np

B, T = 8192, 1024
NCORES = 8
ROWS = B // NCORES          # rows per core
P = 128                     # SBUF partitions
MMCHUNK = 512               # matmul free-dim chunk (one PSUM bank)

_CACHE: dict = {}


def _build_program(rows: int):
    """Build the Bass/Tile program for one core processing `rows` rows."""
    from contextlib import ExitStack

    import concourse.bacc as bacc
    import concourse.mybir as mybir
    import concourse.tile as tile

    f32 = mybir.dt.float32
    f16 = mybir.dt.float16
    i32 = mybir.dt.int32
    Op = mybir.AluOpType

    ablk = int(os.environ.get("K_A", "1"))     # row-groups per sub-block
    nsub = rows // (P * ablk)
    assert nsub * P * ablk == rows

    # Bacc (not raw Bass): its compile pipeline splits multi-sem sync waits
    # into event-semaphore instructions — this walrus build allows at most
    # one wait per engine instruction.
    nc = bacc.Bacc()
    seq = nc.dram_tensor("seq", [rows, T], i32, kind="ExternalInput")
    lp = nc.dram_tensor("lp", [rows, T], f32, kind="ExternalInput")
    val = nc.dram_tensor("val", [rows, T], f32, kind="ExternalInput")
    rew = nc.dram_tensor("rew", [rows, T], f32, kind="ExternalInput")
    out_num = nc.dram_tensor("out_num", [1, MMCHUNK], f32,
                             kind="ExternalOutput")
    out_den = nc.dram_tensor("out_den", [1, MMCHUNK], f32,
                             kind="ExternalOutput")

    def dram_sub(t, r0, na):
        # rows [r0, r0 + na*P) as [p, a, t] with row = r0 + a*P + p
        return t[r0:r0 + na * P, :].rearrange("(a p) t -> p a t", p=P)

    light_tail = bool(int(os.environ.get("K_LIGHT_TAIL", "1")))
    ring_split = bool(int(os.environ.get("K_RING_SPLIT", "0")))

    with ExitStack() as ctx:
        tc = ctx.enter_context(tile.TileContext(nc))
        if light_tail:
            # Replace Tile's end-of-kernel epilogue (drain + two all-engine
            # EVSEM barriers + 64-sem clear, ~8-9us) with just the final
            # drain. Safe for re-execution: the Bass preamble dma_reset +
            # sem_clear runs at the START of every execution, so leaving
            # semaphores dirty at kernel end is fine.
            import types

            from concourse.vector_clock import ScopedClock

            def _light_drain_and_barrier(self, tick_clock, wait_clock):
                drain_inst = self.nc.sync.drain()
                wait_clock.add_sem_waits(
                    drain_inst.ins,
                    ScopedClock({None: tick_clock.global_clock}))
                popped = self.nc._tile_sem_poison_stack.pop()
                assert popped is self._sem_poison
                # Deliberately do NOT free the tile sems: Bacc's
                # event-semaphore pass allocates from the free pool after
                # this and must not alias sems still used by the kernel.

            tc._drain_and_barrier = types.MethodType(
                _light_drain_and_barrier, tc)
        const_pool = ctx.enter_context(tc.tile_pool(name="const", bufs=1))
        # One staging buffer per sub-block: the DMA ring never waits on a
        # buffer free, so the stream runs gap-free at line rate.
        in_pool = ctx.enter_context(tc.tile_pool(name="in", bufs=nsub))
        h_pool = ctx.enter_context(tc.tile_pool(name="h", bufs=2))
        scr_pool = ctx.enter_context(tc.tile_pool(name="scr", bufs=2))
        psum_pool = ctx.enter_context(
            tc.tile_pool(name="psum", bufs=1, space="PSUM"))

        ones16 = const_pool.tile([P, 1], f16)
        nc.vector.memset(ones16[:], 1.0)

        num_ps = psum_pool.tile([1, MMCHUNK], f32)
        den_ps = psum_pool.tile([1, MMCHUNK], f32)

        for si in range(nsub):
            r0 = si * P * ablk
            na = ablk
            seq_t = in_pool.tile([P, na, T], i32, tag="seq")
            lp_t = in_pool.tile([P, na, T], f32, tag="lp")
            val_t = in_pool.tile([P, na, T], f32, tag="val")
            rew_t = in_pool.tile([P, na, T], f32, tag="rew")
            # Single sync HWDGE ring carries the whole input stream (issue
            # order = arrival order); the scalar engine stays free for casts.
            # seq first (the scan is the longest DVE op), then val/rew (d),
            # then lp (q).
            eng2 = nc.scalar if ring_split else nc.sync
            last = si == nsub - 1
            nc.sync.dma_start(out=seq_t[:], in_=dram_sub(seq, r0, na))
            nc.sync.dma_start(out=val_t[:], in_=dram_sub(val, r0, na))
            eng2.dma_start(out=rew_t[:], in_=dram_sub(rew, r0, na))
            # NOTE: never slice the DRAM side along T — a strided DRAM source
            # defeats descriptor coalescing and runs at ~25 GB/s (measured).
            eng2.dma_start(out=lp_t[:], in_=dram_sub(lp, r0, na))

            # f32 -> fp16 casts on the (otherwise idle) Activation engine.
            # The last sub-block's lp is the final DMA of the whole stream:
            # skip its cast (q then reads lp as f32 at 1x) to drop a
            # sem-wait + cast hop from the data-end -> output critical path.
            lp16 = None if last else h_pool.tile([P, na, T], f16, tag="lp16")
            val16 = h_pool.tile([P, na, T], f16, tag="val16")
            rew16 = h_pool.tile([P, na, T], f16, tag="rew16")
            nc.scalar.copy(val16[:], val_t[:])
            nc.scalar.copy(rew16[:], rew_t[:])
            if not last:
                nc.scalar.copy(lp16[:], lp_t[:])

            # mask[p,a,0] = 1; mask[p,a,t] = AND(seq[p,a,0..t-1] != 0)
            mask = scr_pool.tile([P, na, T], f16, tag="mask", bufs=3)
            nc.vector.memset(mask[:, :, 0:1], 1.0)
            for a in range(na):
                nc.vector.tensor_tensor_scan(
                    out=mask[:, a, 1:T], data0=seq_t[:, a, 0:T - 1],
                    data1=seq_t[:, a, 0:T - 1], initial=1.0,
                    op0=Op.logical_and, op1=Op.bypass)

            # den column sums can go to PE as soon as the mask exists.
            for a in range(na):
                for c in range(0, T, MMCHUNK):
                    nc.tensor.matmul(
                        out=den_ps[:], lhsT=ones16[:],
                        rhs=mask[:, a, c:c + MMCHUNK],
                        start=(si == 0 and a == 0 and c == 0),
                        stop=(si == nsub - 1 and a == na - 1
                              and c == T - MMCHUNK))

            # d = val - rew ; dm = d * mask ; mq = logp * dm   (fp16, 2x).
            # Grouping the mask product with d (not with logp) means the
            # lp-dependent work is a single op — lp is always the last tile
            # of a sub-block to arrive, and the very last DMA of the stream.
            d16 = scr_pool.tile([P, na, T], f16, tag="d")
            nc.vector.tensor_tensor(out=d16[:], in0=val16[:], in1=rew16[:],
                                    op=Op.subtract)
            dm = scr_pool.tile([P, na, T], f16, tag="dm")
            nc.vector.tensor_tensor(out=dm[:], in0=d16[:], in1=mask[:],
                                    op=Op.mult)
            mq = scr_pool.tile([P, na, T], f16, tag="mq", bufs=3)
            # Last sub-block reads lp as f32 (1x) — one op after the final
            # DMA of the stream.  Whole-tile writes only: region-sliced tile
            # writes have shown dropped-dependency races.
            mq_src = lp_t if last else lp16
            nc.vector.tensor_tensor(out=mq[:], in0=mq_src[:], in1=dm[:],
                                    op=Op.mult)
            for a in range(na):
                for c in range(0, T, MMCHUNK):
                    nc.tensor.matmul(
                        out=num_ps[:], lhsT=ones16[:],
                        rhs=mq[:, a, c:c + MMCHUNK],
                        start=(si == 0 and a == 0 and c == 0),
                        stop=(si == nsub - 1 and a == na - 1
                              and c == T - MMCHUNK))

        # PSUM can't be DMA'd directly — bounce through SBUF.  den finishes
        # first (its last matmul precedes num's), so its copy + store overlap
        # the num tail; the two copies run on different engines in parallel.
        num_sb = const_pool.tile([1, MMCHUNK], f32)
        den_sb = const_pool.tile([1, MMCHUNK], f32)
        Hc = MMCHUNK // 2
        nc.scalar.copy(den_sb[:], den_ps[:])
        nc.sync.dma_start(out=out_den[:], in_=den_sb[:])
        nc.vector.tensor_copy(num_sb[:, 0:Hc], num_ps[:, 0:Hc])
        nc.scalar.copy(num_sb[:, Hc:], num_ps[:, Hc:])
        nc.sync.dma_start(out=out_num[:], in_=num_sb[:])

    nc.finalize()
    return nc


def kernel(sample_seq, sample_seqLogprobs, sample_value, sample_reward):
    from concourse.bass_utils import run_bass_kernel_spmd

    seq = np.ascontiguousarray(np.asarray(sample_seq, dtype=np.int32))
    lp = np.ascontiguousarray(np.asarray(sample_seqLogprobs, dtype=np.float32))
    val = np.ascontiguousarray(np.asarray(sample_value, dtype=np.float32))
    rew = np.ascontiguousarray(np.asarray(sample_reward, dtype=np.float32))
    assert seq.shape == (B, T)

    if "nc" not in _CACHE:
        _CACHE["nc"] = _build_program(ROWS)
    nc = _CACHE["nc"]

    in_maps = []
    for c in range(NCORES):
        sl = slice(c * ROWS, (c + 1) * ROWS)
        in_maps.append({
            "seq": seq[sl], "lp": lp[sl], "val": val[sl], "rew": rew[sl],
        })

    trace = bool(int(os.environ.get("K_TRACE", "0")))
    res = run_bass_kernel_spmd(nc, in_maps, core_ids=list(range(NCORES)),
                               trace=trace)
    if trace:
        _CACHE["exec_time_ns"] = res.exec_time_ns
        _CACHE["trace"] = res.instructions_and_trace
    num = 0.0
    den = 0.0
    for r in res.results:
        num += float(np.asarray(r["out_num"], dtype=np.float64).sum())
        den += float(np.asarray(r["out_den"], dtype=np.float64).sum())
    return np.float32(num / den)
